# revision 12
# baseline (speedup 1.0000x reference)
# Trainium2 Bass kernel for nn_EssentialToPose.
#
# Pipeline:
#   host (CPU, tiny O(B) work): batched 3x3 SVD of essential_raw + the
#     per-sample pose candidates r1, r2, t.  This must be computed with the
#     same LAPACK the reference uses: ~44% of the samples land on exact
#     chirality-vote ties, where the reference's strict `>` tie-break selects
#     r2/-t, and *which* matrix is "r2" depends on the SVD's per-column sign
#     convention.  Reproducing LAPACK sgesdd's signs on-device is not
#     feasible; everything downstream of the SVD (99.99% of FLOPs / bytes:
#     the B x N x 4-hypothesis triangulation + chirality voting + pose
#     selection + output assembly) runs on the 8 NeuronCores.
#   device (8 cores, batch-parallel, 256 samples/core): for every point n and
#     both rotations, evaluate the two division-free depth signs
#        z1 = alpha - beta*d2z + g*tz        (sign of depth in cam 1 * 2g)
#        z2 = alpha*cd1 - beta - g*ct        (sign of depth in cam 2 * 2g)
#     (g = |p1|^2|p2|^2 - (p1.R p2)^2 >= 0 by Cauchy-Schwarz, so multiplying
#     the reference's divisions through by 2g preserves the strict sign
#     tests), count sign patterns, vote, select pose, assemble [B,4,4].
#
#   Algebraic reductions used on-device:
#     q = R^T t = u[:,:,2]  for BOTH rotations (t = v e3, W^T e3 = W e3 = e3)
#     ct = t . R[:,2] = u22 for both rotations
#     r2 = 2 v2 u2^T - r1   => dot12/d2z/cd1 for r2 are 1 fused op each.

import numpy as np

B, N, NCORES = 2048, 2048, 8
BC = B // NCORES          # samples per core
P = 128                   # SBUF partitions
NTILE = BC // P           # sample tiles per core
NCHUNK = 2                # chunks over the N points
NC = N // NCHUNK

_prog_cache = {}


def _host_prep(essential_raw, c2d2d, k_matrix):
    """CPU-side per-sample prep, mirroring reference.py ops bit-for-bit."""
    import jax
    cpu = jax.devices("cpu")[0]
    import jax.numpy as jnp

    with jax.default_device(cpu):
        E = jnp.asarray(np.ascontiguousarray(np.asarray(essential_raw)))
        U, S, Vh = jnp.linalg.svd(E, full_matrices=False)
        V = jnp.swapaxes(Vh, 1, 2)
        u = U * jnp.sign(jnp.linalg.det(U))[:, None, None]
        v = V * jnp.sign(jnp.linalg.det(V))[:, None, None]
        last_svs = S[:, -1]

        W = jnp.array([[0.0, -1.0, 0.0], [1.0, 0.0, 0.0], [0.0, 0.0, 1.0]],
                      dtype=jnp.float32)
        ut = jnp.swapaxes(u, 1, 2)
        r1 = v @ W @ ut
        r2 = v @ W.T @ ut
        t = v[:, :, 2]
        # reference normalizes after sign selection; (-t)/||t|| == -(t/||t||)
        # exactly in IEEE, so the device just flips the sign of t_unit.
        t_unit = t / jnp.sqrt((t ** 2).sum(axis=-1, keepdims=True))

        k_inv = jnp.linalg.inv(jnp.asarray(np.asarray(k_matrix)))

        r1 = np.asarray(r1); r2 = np.asarray(r2)
        t = np.asarray(t); un = np.asarray(u)
        t_unit = np.asarray(t_unit)
        last_svs = np.asarray(last_svs)
        k_inv = np.asarray(k_inv)

    q = un[:, :, 2]                      # R^T t for both rotations
    coef = np.zeros((B, 32), np.float32)
    coef[:, 0:9] = r1.reshape(B, 9)
    coef[:, 9:18] = r2.reshape(B, 9)
    coef[:, 18:21] = t
    coef[:, 21:24] = q
    coef[:, 24] = np.float32(2.0) * t[:, 2]     # 2*tz   (d2z_r2 fold)
    coef[:, 25] = np.float32(2.0) * q[:, 2]     # 2*ct   (cd1_r2 fold)
    coef[:, 26] = -q[:, 2]                      # -ct    (z2 fused scale)
    coef[:, 27:30] = t_unit

    kc = (float(k_inv[0, 0]), float(k_inv[0, 2]),
          float(k_inv[1, 1]), float(k_inv[1, 2]))
    return coef, last_svs, kc


def _emit(tc, pts, coef, tf, kc):
    from contextlib import ExitStack
    import concourse.bass as bass  # noqa: F401
    from concourse import mybir

    nc = tc.nc
    f32 = mybir.dt.float32
    A = mybir.AluOpType
    AF = mybir.ActivationFunctionType
    ka0, kb0, ka1, kb1 = kc

    with ExitStack() as ctx:
        io = ctx.enter_context(tc.tile_pool(name="io", bufs=2))
        cpool = ctx.enter_context(tc.tile_pool(name="cpool", bufs=2))
        fp = ctx.enter_context(tc.tile_pool(name="fp", bufs=1))
        sp = ctx.enter_context(tc.tile_pool(name="sp", bufs=2))

        neg1 = cpool.tile([P, 1], f32, tag="neg1", name="neg1", bufs=1)
        nc.vector.memset(neg1, -1.0)

        for it in range(NTILE):
            s0 = it * P
            cf = cpool.tile([P, 32], f32, tag="cf", name="cf")
            nc.sync.dma_start(out=cf, in_=coef[s0:s0 + P, :])

            def C(i):
                return cf[:, i:i + 1]

            # per-chunk partial counts: [r1t | r1m | r2t | r2m] x NCHUNK cols
            cnts = sp.tile([P, 4 * NCHUNK], f32, tag="cnts", name="cnts")

            for k in range(NCHUNK):
                def T(tag):
                    return fp.tile([P, NC], f32, tag=tag, name=tag)

                blk = io.tile([P, NC, 4], f32, tag="blk", name="blk")
                nc.sync.dma_start(out=blk, in_=pts[s0:s0 + P, k * NC:(k + 1) * NC, :])

                # normalized ray xy components (ACT, strided read)
                p1x = T("p1x"); p1y = T("p1y"); p2x = T("p2x"); p2y = T("p2y")
                nc.scalar.activation(p1x, blk[:, :, 0], AF.Copy, bias=kb0, scale=ka0)
                nc.scalar.activation(p1y, blk[:, :, 1], AF.Copy, bias=kb1, scale=ka1)
                nc.scalar.activation(p2x, blk[:, :, 2], AF.Copy, bias=kb0, scale=ka0)
                nc.scalar.activation(p2y, blk[:, :, 3], AF.Copy, bias=kb1, scale=ka1)

                # |p1|^2, |p2|^2, product
                sq1 = T("sq1"); sq2 = T("sq2"); sq3 = T("sq3"); sq4 = T("sq4")
                nc.vector.tensor_mul(sq1, p1x, p1x)
                nc.vector.tensor_mul(sq2, p1y, p1y)
                nc.vector.tensor_mul(sq3, p2x, p2x)
                nc.vector.tensor_mul(sq4, p2y, p2y)
                d11 = T("d11"); d22 = T("d22"); dd = T("dd")
                nc.vector.scalar_tensor_tensor(d11, sq2, 1.0, sq1, A.add, A.add)
                nc.vector.scalar_tensor_tensor(d22, sq4, 1.0, sq3, A.add, A.add)
                nc.gpsimd.tensor_mul(dd, d11, d22)

                # outer products
                oa = T("oa"); ob = T("ob"); oc = T("oc"); od = T("od")
                nc.vector.tensor_mul(oa, p1x, p2x)
                nc.vector.tensor_mul(ob, p1x, p2y)
                nc.vector.tensor_mul(oc, p1y, p2x)
                nc.vector.tensor_mul(od, p1y, p2y)

                # t.p1 and q.p2
                tp1 = T("tp1"); qp2 = T("qp2")
                nc.vector.tensor_scalar(tp1, p1x, C(18), C(20), A.mult, A.add)
                nc.vector.scalar_tensor_tensor(tp1, p1y, C(19), tp1, A.mult, A.add)
                nc.vector.tensor_scalar(qp2, p2x, C(21), C(23), A.mult, A.add)
                nc.vector.scalar_tensor_tensor(qp2, p2y, C(22), qp2, A.mult, A.add)

                X = T("X"); Y = T("Y"); tq = T("tq")
                nc.gpsimd.tensor_mul(X, tp1, d22)
                nc.gpsimd.tensor_mul(Y, qp2, d11)
                nc.gpsimd.tensor_mul(tq, tp1, qp2)

                # R1 linear forms.  cd1 = R02 p1x + R12 p1y + R22 and
                # d2z = R20 p2x + R21 p2y + R22 are sub-expressions of
                # dot12 = p1^T r1 p2: dot12 = cd1 + d2z - R22 + sum(outer*R).
                # Outer-product (gpsimd) terms last so the DVE stream never
                # head-of-line blocks on Pool results.
                d12a = T("d12a"); d2za = T("d2za"); cd1a = T("cd1a")
                nc.vector.tensor_scalar(d2za, p2x, C(6), C(8), A.mult, A.add)
                nc.vector.scalar_tensor_tensor(d2za, p2y, C(7), d2za, A.mult, A.add)
                nc.vector.tensor_scalar(cd1a, p1x, C(2), C(8), A.mult, A.add)
                nc.vector.scalar_tensor_tensor(cd1a, p1y, C(5), cd1a, A.mult, A.add)
                wsum = T("wsum")
                nc.vector.scalar_tensor_tensor(wsum, cd1a, C(8), d2za,
                                               A.subtract, A.add)
                nc.vector.scalar_tensor_tensor(d12a, oa, C(0), wsum, A.mult, A.add)
                for feat, ci in ((ob, 1), (oc, 3), (od, 4)):
                    nc.vector.scalar_tensor_tensor(d12a, feat, C(ci), d12a,
                                                   A.mult, A.add)

                # R2 forms via r2 = 2 v2 u2^T - r1 (fresh tiles: R1/R2 overlap)
                d12b = T("d12b"); d2zb = T("d2zb"); cd1b = T("cd1b")
                nc.vector.scalar_tensor_tensor(d12b, tq, 2.0, d12a, A.mult, A.subtract)
                nc.vector.scalar_tensor_tensor(d2zb, qp2, C(24), d2za, A.mult, A.subtract)
                nc.vector.scalar_tensor_tensor(cd1b, tp1, C(25), cd1a, A.mult, A.subtract)

                # two z-blocks, ops interleaved for cross-engine latency hiding
                gsA = T("gsa"); gA = T("ga"); gsB = T("gsb"); gB = T("gb")
                u1A = T("sq1"); u2A = T("sq2"); u1B = T("sq3"); u2B = T("sq4")
                alA = T("ala"); beA = T("bea"); alB = T("alb"); beB = T("beb")
                z1A = T("oa"); z2A = T("ob"); z1B = T("oc"); z2B = T("od")
                s1A = T("p1x"); s2A = T("p1y"); s1B = T("p2x"); s2B = T("p2y")
                ssA = T("ssa"); indA = T("inda"); ssB = T("ssb"); indB = T("indb")

                nc.scalar.square(gsA, d12a)
                nc.scalar.square(gsB, d12b)
                nc.gpsimd.tensor_sub(gA, dd, gsA)
                nc.gpsimd.tensor_sub(gB, dd, gsB)
                nc.vector.tensor_mul(u1A, qp2, d12a)
                nc.vector.tensor_mul(u1B, qp2, d12b)
                nc.vector.tensor_sub(alA, X, u1A)
                nc.vector.tensor_sub(alB, X, u1B)
                nc.vector.tensor_mul(u2A, tp1, d12a)
                nc.vector.tensor_mul(u2B, tp1, d12b)
                nc.vector.tensor_sub(beA, Y, u2A)
                nc.vector.tensor_sub(beB, Y, u2B)
                # z1 = al - be*d2z + g*tz
                nc.vector.tensor_mul(u1A, beA, d2za)
                nc.vector.tensor_mul(u1B, beB, d2zb)
                nc.vector.tensor_sub(u2A, alA, u1A)
                nc.vector.tensor_sub(u2B, alB, u1B)
                nc.vector.scalar_tensor_tensor(z1A, gA, C(20), u2A, A.mult, A.add)
                nc.vector.scalar_tensor_tensor(z1B, gB, C(20), u2B, A.mult, A.add)
                # z2 = al*cd1 - be - g*ct
                nc.vector.tensor_mul(u1A, alA, cd1a)
                nc.vector.tensor_mul(u1B, alB, cd1b)
                nc.vector.tensor_sub(u2A, u1A, beA)
                nc.vector.tensor_sub(u2B, u1B, beB)
                nc.vector.scalar_tensor_tensor(z2A, gA, C(26), u2A, A.mult, A.add)
                nc.vector.scalar_tensor_tensor(z2B, gB, C(26), u2B, A.mult, A.add)

                # counts: s = sign(z1)+sign(z2); both>0 <=> s==2, both<0 <=> s==-2
                nc.scalar.activation(s1A, z1A, AF.Sign)
                nc.scalar.activation(s2A, z2A, AF.Sign)
                nc.scalar.activation(s1B, z1B, AF.Sign)
                nc.scalar.activation(s2B, z2B, AF.Sign)
                nc.gpsimd.tensor_add(ssA, s1A, s2A)
                nc.gpsimd.tensor_add(ssB, s1B, s2B)
                for ss, ind, ct_t, cm_t in ((ssA, indA, 0, 1), (ssB, indB, 2, 3)):
                    nc.scalar.activation(ind, ss, AF.Relu, bias=neg1, scale=1.0,
                                         accum_out=cnts[:, ct_t * NCHUNK + k:
                                                        ct_t * NCHUNK + k + 1])
                    nc.scalar.activation(ind, ss, AF.Relu, bias=neg1, scale=-1.0,
                                         accum_out=cnts[:, cm_t * NCHUNK + k:
                                                        cm_t * NCHUNK + k + 1])

            # ---- votes + pose selection + assembly ----
            sc = sp.tile([P, 4], f32, tag="sc", name="sc")
            for c in range(4):
                nc.vector.reduce_sum(sc[:, c:c + 1],
                                     cnts[:, c * NCHUNK:(c + 1) * NCHUNK],
                                     axis=mybir.AxisListType.X)
            vr1 = sp.tile([P, 1], f32, tag="vr1", name="vr1"); vr2 = sp.tile([P, 1], f32, tag="vr2", name="vr2")
            vt1 = sp.tile([P, 1], f32, tag="vt1", name="vt1"); vt2 = sp.tile([P, 1], f32, tag="vt2", name="vt2")
            nc.vector.tensor_add(vr1, sc[:, 0:1], sc[:, 1:2])
            nc.vector.tensor_add(vr2, sc[:, 2:3], sc[:, 3:4])
            nc.vector.tensor_add(vt1, sc[:, 0:1], sc[:, 2:3])
            nc.vector.tensor_add(vt2, sc[:, 1:2], sc[:, 3:4])
            mask_r = sp.tile([P, 1], f32, tag="mask_r", name="mask_r")
            sgn_t = sp.tile([P, 1], f32, tag="sgn_t", name="sgn_t")
            nc.vector.tensor_tensor(mask_r, vr1, vr2, A.is_gt)
            nc.vector.tensor_tensor(sgn_t, vt1, vt2, A.is_gt)
            nc.vector.tensor_scalar(sgn_t, sgn_t, 2.0, -1.0, A.mult, A.add)

            rdiff = sp.tile([P, 9], f32, tag="rdiff", name="rdiff")
            nc.vector.tensor_sub(rdiff, cf[:, 0:9], cf[:, 9:18])

            tft = sp.tile([P, 16], f32, tag="tft", name="tft")
            nc.vector.memset(tft[:, 12:16], 0.0)
            nc.vector.memset(tft[:, 15:16], 1.0)
            tft4 = tft.rearrange("p (i j) -> p i j", j=4)
            r_view = tft4[:, 0:3, 0:3]
            t_view = tft4[:, 0:3, 3]
            nc.vector.scalar_tensor_tensor(
                r_view, rdiff.rearrange("p (i j) -> p i j", j=3), mask_r,
                cf[:, 9:18].rearrange("p (i j) -> p i j", j=3), A.mult, A.add)
            nc.vector.tensor_scalar(t_view, cf[:, 27:30], sgn_t, None, A.mult)

            nc.sync.dma_start(out=tf[s0:s0 + P, :], in_=tft)


def _build_program(kc):
    key = kc
    if key in _prog_cache:
        return _prog_cache[key]
    import concourse.tile as tile
    from concourse import bacc, mybir

    nc = bacc.Bacc("TRN2", target_bir_lowering=False, debug=False,
                   num_devices=NCORES)
    pts = nc.dram_tensor("pts", [BC, N, 4], mybir.dt.float32,
                         kind="ExternalInput").ap()
    coef = nc.dram_tensor("coef", [BC, 32], mybir.dt.float32,
                          kind="ExternalInput").ap()
    tf = nc.dram_tensor("tf", [BC, 16], mybir.dt.float32,
                        kind="ExternalOutput").ap()
    with tile.TileContext(nc) as tc:
        _emit(tc, pts, coef, tf, kc)
    nc.compile()
    _prog_cache[key] = nc
    return nc


def kernel(essential_raw, c2d2d, k_matrix):
    from concourse import bass_utils

    essential_raw = np.asarray(essential_raw, np.float32)
    c2d2d = np.ascontiguousarray(np.asarray(c2d2d, np.float32))
    k_matrix = np.asarray(k_matrix, np.float32)

    coef, last_svs, kc = _host_prep(essential_raw, c2d2d, k_matrix)
    nc = _build_program(kc)

    in_maps = [
        {"pts": c2d2d[i * BC:(i + 1) * BC], "coef": coef[i * BC:(i + 1) * BC]}
        for i in range(NCORES)
    ]
    res = bass_utils.run_bass_kernel_spmd(nc, in_maps, core_ids=list(range(NCORES)))
    tf = np.concatenate([np.asarray(r["tf"]) for r in res.results], axis=0)
    return tf.reshape(B, 4, 4), last_svs


# revision 13
# speedup vs baseline: 1.1961x; 1.1961x over previous
# Trainium2 Bass kernel for nn_EssentialToPose.
#
# Pipeline:
#   host (CPU, tiny O(B) work): batched 3x3 SVD of essential_raw + the
#     per-sample pose candidates r1, r2, t.  This must be computed with the
#     same LAPACK the reference uses: ~44% of the samples land on exact
#     chirality-vote ties, where the reference's strict `>` tie-break selects
#     r2/-t, and *which* matrix is "r2" depends on the SVD's per-column sign
#     convention.  Reproducing LAPACK sgesdd's signs on-device is not
#     feasible; everything downstream of the SVD (99.99% of FLOPs / bytes:
#     the B x N x 4-hypothesis triangulation + chirality voting + pose
#     selection + output assembly) runs on the 8 NeuronCores.
#   device (8 cores, batch-parallel, 256 samples/core): for every point n and
#     both rotations, evaluate the two division-free depth signs
#        z1 = alpha - beta*d2z + g*tz        (sign of depth in cam 1 * 2g)
#        z2 = alpha*cd1 - beta - g*ct        (sign of depth in cam 2 * 2g)
#     (g = |p1|^2|p2|^2 - (p1.R p2)^2 >= 0 by Cauchy-Schwarz, so multiplying
#     the reference's divisions through by 2g preserves the strict sign
#     tests), count sign patterns, vote, select pose, assemble [B,4,4].
#
#   Algebraic reductions used on-device:
#     q = R^T t = u[:,:,2]  for BOTH rotations (t = v e3, W^T e3 = W e3 = e3)
#     ct = t . R[:,2] = u22 for both rotations
#     r2 = 2 v2 u2^T - r1   => dot12/d2z/cd1 for r2 are 1 fused op each.

import numpy as np

B, N, NCORES = 2048, 2048, 8
BC = B // NCORES          # samples per core
P = 128                   # SBUF partitions
NTILE = BC // P           # sample tiles per core
NCHUNK = 2                # chunks over the N points
NC = N // NCHUNK

_prog_cache = {}


def _host_prep(essential_raw, c2d2d, k_matrix):
    """CPU-side per-sample prep, mirroring reference.py ops bit-for-bit."""
    import jax
    cpu = jax.devices("cpu")[0]
    import jax.numpy as jnp

    with jax.default_device(cpu):
        E = jnp.asarray(np.ascontiguousarray(np.asarray(essential_raw)))
        U, S, Vh = jnp.linalg.svd(E, full_matrices=False)
        V = jnp.swapaxes(Vh, 1, 2)
        u = U * jnp.sign(jnp.linalg.det(U))[:, None, None]
        v = V * jnp.sign(jnp.linalg.det(V))[:, None, None]
        last_svs = S[:, -1]

        W = jnp.array([[0.0, -1.0, 0.0], [1.0, 0.0, 0.0], [0.0, 0.0, 1.0]],
                      dtype=jnp.float32)
        ut = jnp.swapaxes(u, 1, 2)
        r1 = v @ W @ ut
        r2 = v @ W.T @ ut
        t = v[:, :, 2]
        # reference normalizes after sign selection; (-t)/||t|| == -(t/||t||)
        # exactly in IEEE, so the device just flips the sign of t_unit.
        t_unit = t / jnp.sqrt((t ** 2).sum(axis=-1, keepdims=True))

        k_inv = jnp.linalg.inv(jnp.asarray(np.asarray(k_matrix)))

        r1 = np.asarray(r1); r2 = np.asarray(r2)
        t = np.asarray(t); un = np.asarray(u)
        t_unit = np.asarray(t_unit)
        last_svs = np.asarray(last_svs)
        k_inv = np.asarray(k_inv)

    q = un[:, :, 2]                      # R^T t for both rotations
    coef = np.zeros((B, 32), np.float32)
    coef[:, 0:9] = r1.reshape(B, 9)
    coef[:, 9:18] = r2.reshape(B, 9)
    coef[:, 18:21] = t
    coef[:, 21:24] = q
    coef[:, 24] = np.float32(2.0) * t[:, 2]     # 2*tz   (d2z_r2 fold)
    coef[:, 25] = np.float32(2.0) * q[:, 2]     # 2*ct   (cd1_r2 fold)
    coef[:, 26] = -q[:, 2]                      # -ct    (z2 fused scale)
    coef[:, 27:30] = t_unit

    kc = (float(k_inv[0, 0]), float(k_inv[0, 2]),
          float(k_inv[1, 1]), float(k_inv[1, 2]))
    return coef, last_svs, kc


def _emit(tc, pts, coef, tf, kc):
    from contextlib import ExitStack
    import concourse.bass as bass  # noqa: F401
    from concourse import mybir

    nc = tc.nc
    f32 = mybir.dt.float32
    A = mybir.AluOpType
    AF = mybir.ActivationFunctionType
    ka0, kb0, ka1, kb1 = kc

    with ExitStack() as ctx:
        io = ctx.enter_context(tc.tile_pool(name="io", bufs=2))
        cpool = ctx.enter_context(tc.tile_pool(name="cpool", bufs=2))
        fp = ctx.enter_context(tc.tile_pool(name="fp", bufs=1))
        sp = ctx.enter_context(tc.tile_pool(name="sp", bufs=2))

        neg1 = cpool.tile([P, 1], f32, tag="neg1", name="neg1", bufs=1)
        nc.vector.memset(neg1, -1.0)

        for it in range(NTILE):
            s0 = it * P
            cf = cpool.tile([P, 32], f32, tag="cf", name="cf")
            nc.sync.dma_start(out=cf, in_=coef[s0:s0 + P, :])

            def C(i):
                return cf[:, i:i + 1]

            # per-chunk partial counts: [r1t | r1m | r2t | r2m] x NCHUNK cols
            cnts = sp.tile([P, 4 * NCHUNK], f32, tag="cnts", name="cnts")

            for k in range(NCHUNK):
                def T(tag):
                    return fp.tile([P, NC], f32, tag=tag, name=tag)

                blk = io.tile([P, NC, 4], f32, tag="blk", name="blk")
                nc.sync.dma_start(out=blk, in_=pts[s0:s0 + P, k * NC:(k + 1) * NC, :])

                # normalized ray xy components (ACT, strided read)
                p1x = T("p1x"); p1y = T("p1y"); p2x = T("p2x"); p2y = T("p2y")
                nc.scalar.activation(p1x, blk[:, :, 0], AF.Copy, bias=kb0, scale=ka0)
                nc.scalar.activation(p1y, blk[:, :, 1], AF.Copy, bias=kb1, scale=ka1)
                nc.scalar.activation(p2x, blk[:, :, 2], AF.Copy, bias=kb0, scale=ka0)
                nc.scalar.activation(p2y, blk[:, :, 3], AF.Copy, bias=kb1, scale=ka1)

                # |p1|^2, |p2|^2, product
                sq1 = T("sq1"); sq2 = T("sq2"); sq3 = T("sq3"); sq4 = T("sq4")
                nc.scalar.square(sq1, p1x)
                nc.scalar.square(sq2, p1y)
                nc.scalar.square(sq3, p2x)
                nc.scalar.square(sq4, p2y)
                d11 = T("d11"); d22 = T("d22"); dd = T("dd")
                nc.vector.scalar_tensor_tensor(d11, sq2, 1.0, sq1, A.add, A.add)
                nc.vector.scalar_tensor_tensor(d22, sq4, 1.0, sq3, A.add, A.add)
                nc.vector.tensor_mul(dd, d11, d22)

                # outer products
                oa = T("oa"); ob = T("ob"); oc = T("oc"); od = T("od")
                nc.vector.tensor_mul(oa, p1x, p2x)
                nc.vector.tensor_mul(ob, p1x, p2y)
                nc.vector.tensor_mul(oc, p1y, p2x)
                nc.vector.tensor_mul(od, p1y, p2y)

                # t.p1 and q.p2
                tp1 = T("tp1"); qp2 = T("qp2")
                nc.vector.tensor_scalar(tp1, p1x, C(18), C(20), A.mult, A.add)
                nc.vector.scalar_tensor_tensor(tp1, p1y, C(19), tp1, A.mult, A.add)
                nc.vector.tensor_scalar(qp2, p2x, C(21), C(23), A.mult, A.add)
                nc.vector.scalar_tensor_tensor(qp2, p2y, C(22), qp2, A.mult, A.add)

                X = T("X"); Y = T("Y"); tq = T("tq")
                nc.vector.tensor_mul(X, tp1, d22)
                nc.vector.tensor_mul(Y, qp2, d11)
                nc.vector.tensor_mul(tq, tp1, qp2)

                # R1 linear forms.  cd1 = R02 p1x + R12 p1y + R22 and
                # d2z = R20 p2x + R21 p2y + R22 are sub-expressions of
                # dot12 = p1^T r1 p2: dot12 = cd1 + d2z - R22 + sum(outer*R).
                # Outer-product (gpsimd) terms last so the DVE stream never
                # head-of-line blocks on Pool results.
                d12a = T("d12a"); d2za = T("d2za"); cd1a = T("cd1a")
                nc.vector.tensor_scalar(d2za, p2x, C(6), C(8), A.mult, A.add)
                nc.vector.scalar_tensor_tensor(d2za, p2y, C(7), d2za, A.mult, A.add)
                nc.vector.tensor_scalar(cd1a, p1x, C(2), C(8), A.mult, A.add)
                nc.vector.scalar_tensor_tensor(cd1a, p1y, C(5), cd1a, A.mult, A.add)
                wsum = T("wsum")
                nc.vector.scalar_tensor_tensor(wsum, cd1a, C(8), d2za,
                                               A.subtract, A.add)
                nc.vector.scalar_tensor_tensor(d12a, oa, C(0), wsum, A.mult, A.add)
                for feat, ci in ((ob, 1), (oc, 3), (od, 4)):
                    nc.vector.scalar_tensor_tensor(d12a, feat, C(ci), d12a,
                                                   A.mult, A.add)

                # R2 forms via r2 = 2 v2 u2^T - r1 (fresh tiles: R1/R2 overlap)
                d12b = T("d12b"); d2zb = T("d2zb"); cd1b = T("cd1b")
                nc.vector.scalar_tensor_tensor(d12b, tq, 2.0, d12a, A.mult, A.subtract)
                nc.vector.scalar_tensor_tensor(d2zb, qp2, C(24), d2za, A.mult, A.subtract)
                nc.vector.scalar_tensor_tensor(cd1b, tp1, C(25), cd1a, A.mult, A.subtract)

                # two z-blocks, ops interleaved for cross-engine latency hiding
                gsA = T("gsa"); gA = T("ga"); gsB = T("gsb"); gB = T("gb")
                u1A = T("sq1"); u2A = T("sq2"); u1B = T("sq3"); u2B = T("sq4")
                alA = T("ala"); beA = T("bea"); alB = T("alb"); beB = T("beb")
                z1A = T("oa"); z2A = T("ob"); z1B = T("oc"); z2B = T("od")
                s1A = T("p1x"); s2A = T("p1y"); s1B = T("p2x"); s2B = T("p2y")
                ssA = T("ssa"); indA = T("inda"); ssB = T("ssb"); indB = T("indb")

                nc.scalar.square(gsA, d12a)
                nc.scalar.square(gsB, d12b)
                nc.vector.tensor_sub(gA, dd, gsA)
                nc.vector.tensor_sub(gB, dd, gsB)
                nc.vector.tensor_mul(u1A, qp2, d12a)
                nc.vector.tensor_mul(u1B, qp2, d12b)
                nc.vector.tensor_sub(alA, X, u1A)
                nc.vector.tensor_sub(alB, X, u1B)
                nc.vector.tensor_mul(u2A, tp1, d12a)
                nc.vector.tensor_mul(u2B, tp1, d12b)
                nc.vector.tensor_sub(beA, Y, u2A)
                nc.vector.tensor_sub(beB, Y, u2B)
                # z1 = al - be*d2z + g*tz
                nc.vector.tensor_mul(u1A, beA, d2za)
                nc.vector.tensor_mul(u1B, beB, d2zb)
                nc.vector.tensor_sub(u2A, alA, u1A)
                nc.vector.tensor_sub(u2B, alB, u1B)
                nc.vector.scalar_tensor_tensor(z1A, gA, C(20), u2A, A.mult, A.add)
                nc.vector.scalar_tensor_tensor(z1B, gB, C(20), u2B, A.mult, A.add)
                # z2 = al*cd1 - be - g*ct
                nc.vector.tensor_mul(u1A, alA, cd1a)
                nc.vector.tensor_mul(u1B, alB, cd1b)
                nc.vector.tensor_sub(u2A, u1A, beA)
                nc.vector.tensor_sub(u2B, u1B, beB)
                nc.vector.scalar_tensor_tensor(z2A, gA, C(26), u2A, A.mult, A.add)
                nc.vector.scalar_tensor_tensor(z2B, gB, C(26), u2B, A.mult, A.add)

                # counts: s = sign(z1)+sign(z2); both>0 <=> s==2, both<0 <=> s==-2
                nc.scalar.activation(s1A, z1A, AF.Sign)
                nc.scalar.activation(s2A, z2A, AF.Sign)
                nc.scalar.activation(s1B, z1B, AF.Sign)
                nc.scalar.activation(s2B, z2B, AF.Sign)
                nc.vector.tensor_add(ssA, s1A, s2A)
                nc.vector.tensor_add(ssB, s1B, s2B)
                for ss, ind, ct_t, cm_t in ((ssA, indA, 0, 1), (ssB, indB, 2, 3)):
                    nc.scalar.activation(ind, ss, AF.Relu, bias=neg1, scale=1.0,
                                         accum_out=cnts[:, ct_t * NCHUNK + k:
                                                        ct_t * NCHUNK + k + 1])
                    nc.scalar.activation(ind, ss, AF.Relu, bias=neg1, scale=-1.0,
                                         accum_out=cnts[:, cm_t * NCHUNK + k:
                                                        cm_t * NCHUNK + k + 1])

            # ---- votes + pose selection + assembly ----
            sc = sp.tile([P, 4], f32, tag="sc", name="sc")
            for c in range(4):
                nc.vector.reduce_sum(sc[:, c:c + 1],
                                     cnts[:, c * NCHUNK:(c + 1) * NCHUNK],
                                     axis=mybir.AxisListType.X)
            vr1 = sp.tile([P, 1], f32, tag="vr1", name="vr1"); vr2 = sp.tile([P, 1], f32, tag="vr2", name="vr2")
            vt1 = sp.tile([P, 1], f32, tag="vt1", name="vt1"); vt2 = sp.tile([P, 1], f32, tag="vt2", name="vt2")
            nc.vector.tensor_add(vr1, sc[:, 0:1], sc[:, 1:2])
            nc.vector.tensor_add(vr2, sc[:, 2:3], sc[:, 3:4])
            nc.vector.tensor_add(vt1, sc[:, 0:1], sc[:, 2:3])
            nc.vector.tensor_add(vt2, sc[:, 1:2], sc[:, 3:4])
            mask_r = sp.tile([P, 1], f32, tag="mask_r", name="mask_r")
            sgn_t = sp.tile([P, 1], f32, tag="sgn_t", name="sgn_t")
            nc.vector.tensor_tensor(mask_r, vr1, vr2, A.is_gt)
            nc.vector.tensor_tensor(sgn_t, vt1, vt2, A.is_gt)
            nc.vector.tensor_scalar(sgn_t, sgn_t, 2.0, -1.0, A.mult, A.add)

            rdiff = sp.tile([P, 9], f32, tag="rdiff", name="rdiff")
            nc.vector.tensor_sub(rdiff, cf[:, 0:9], cf[:, 9:18])

            tft = sp.tile([P, 16], f32, tag="tft", name="tft")
            nc.vector.memset(tft[:, 12:16], 0.0)
            nc.vector.memset(tft[:, 15:16], 1.0)
            tft4 = tft.rearrange("p (i j) -> p i j", j=4)
            r_view = tft4[:, 0:3, 0:3]
            t_view = tft4[:, 0:3, 3]
            nc.vector.scalar_tensor_tensor(
                r_view, rdiff.rearrange("p (i j) -> p i j", j=3), mask_r,
                cf[:, 9:18].rearrange("p (i j) -> p i j", j=3), A.mult, A.add)
            nc.vector.tensor_scalar(t_view, cf[:, 27:30], sgn_t, None, A.mult)

            nc.sync.dma_start(out=tf[s0:s0 + P, :], in_=tft)


def _build_program(kc):
    key = kc
    if key in _prog_cache:
        return _prog_cache[key]
    import concourse.tile as tile
    from concourse import bacc, mybir

    nc = bacc.Bacc("TRN2", target_bir_lowering=False, debug=False,
                   num_devices=NCORES)
    pts = nc.dram_tensor("pts", [BC, N, 4], mybir.dt.float32,
                         kind="ExternalInput").ap()
    coef = nc.dram_tensor("coef", [BC, 32], mybir.dt.float32,
                          kind="ExternalInput").ap()
    tf = nc.dram_tensor("tf", [BC, 16], mybir.dt.float32,
                        kind="ExternalOutput").ap()
    with tile.TileContext(nc) as tc:
        _emit(tc, pts, coef, tf, kc)
    nc.compile()
    _prog_cache[key] = nc
    return nc


def kernel(essential_raw, c2d2d, k_matrix):
    from concourse import bass_utils

    essential_raw = np.asarray(essential_raw, np.float32)
    c2d2d = np.ascontiguousarray(np.asarray(c2d2d, np.float32))
    k_matrix = np.asarray(k_matrix, np.float32)

    coef, last_svs, kc = _host_prep(essential_raw, c2d2d, k_matrix)
    nc = _build_program(kc)

    in_maps = [
        {"pts": c2d2d[i * BC:(i + 1) * BC], "coef": coef[i * BC:(i + 1) * BC]}
        for i in range(NCORES)
    ]
    res = bass_utils.run_bass_kernel_spmd(nc, in_maps, core_ids=list(range(NCORES)))
    tf = np.concatenate([np.asarray(r["tf"]) for r in res.results], axis=0)
    return tf.reshape(B, 4, 4), last_svs
